# revision 16
# baseline (speedup 1.0000x reference)
"""Causal self-attention (B=4, T=2048, C=768, 12 heads) on 8 Trainium2 cores.

Sharding: core i handles batch b = i//2 and head-set s = i%2 (6 of 12 heads).
Each core computes x[b] @ W_attn slice -> 6 heads of causal attention -> a
partial projection (row-sharded W_proj).  The host sums the two partials per
batch and adds b_proj.

v2: software-pipelined emission.  The kernel is organized as 4 k-column
groups j=0..3 (512 T-positions each).  Stage A(j) computes kt/qt column j
and V chunks 4j..4j+3; stage B(j) runs causal attention for q-chunk j over
k-chunks 0..4j+3; stage C(j) projects q-chunk j.  A(j+1) and C(j-1) matmul
groups are interleaved into B(j)'s kc loop as PE filler so the ScalarE exp
stream (the bottleneck engine, ~1.1us per kc tile) is never the only
runnable work: PE executes its queue in order, so emission order is the
schedule.

All matmul operands are bf16 (FWL-enabled weight loads, half SBUF/DMA
traffic); PSUM accumulation stays fp32.  Per-head layout as v1: Q^T/K^T in
[128, T] "pair" tiles (head a on partitions 0-63, head b on 64-127,
1/sqrt(64) folded into W_q/b_q), V' [T, 6*65] with a ones-column per head so
the PV matmul emits the softmax denominator row, S^T per (pair, q-chunk 512,
k-chunk 128) via row-tiled concurrent 64-contraction matmuls, exp on
ScalarE straight out of PSUM, diagonal tiles shrunk to live columns
(s0 = 128*m) with a single [128,128] triangle mask multiply on VectorE.
Normalization uses the fast approximate DVE reciprocal (~5x the ~3.3us
exact op).
"""

import numpy as np

import concourse.bass as bass
import concourse.mybir as mybir
import concourse.tile as tile
from concourse import bacc

B, T, C = 4, 2048, 768
NH, HD = 12, 64
N_CORES = 8
HPC = 6  # heads per core
P = 128
F32 = mybir.dt.float32
F32R = mybir.dt.float32r
BF16 = mybir.dt.bfloat16
QC_N = T // 512  # 4 q-chunks of 512
KC_N = T // P    # 16 k-chunks of 128
CKC = C // P     # 6 contraction chunks for the QKV projection


def build_program(n_iters: int = 1, **_compat):
    """Builds the SPMD program (identical on all cores; data differs)."""
    nc = bacc.Bacc(
        "TRN2",
        target_bir_lowering=False,
        debug=False,
        enable_asserts=False,
        num_devices=N_CORES,
    )
    d_xt = nc.dram_tensor("xt", [C, T], BF16, kind="ExternalInput").ap()
    d_wq = nc.dram_tensor("wq", [C, 384], BF16, kind="ExternalInput").ap()
    d_wk = nc.dram_tensor("wk", [C, 384], BF16, kind="ExternalInput").ap()
    d_wv = nc.dram_tensor("wv", [C, 390], BF16, kind="ExternalInput").ap()
    d_w2 = nc.dram_tensor("w2", [384, C], BF16, kind="ExternalInput").ap()
    d_bq = nc.dram_tensor("bq", [P, 3], F32, kind="ExternalInput").ap()
    d_bk = nc.dram_tensor("bk", [P, 3], F32, kind="ExternalInput").ap()
    d_bvb = nc.dram_tensor("bvb", [P, 390], BF16, kind="ExternalInput").ap()
    d_mask = nc.dram_tensor("masks", [P, P], BF16, kind="ExternalInput").ap()
    d_out = nc.dram_tensor("out", [T, C], F32R, kind="ExternalOutput").ap()

    with tile.TileContext(nc) as tc:
        # PSUM budget (8 banks):
        #   tag "ps_A" [128,1024] x2 = 4 banks  (S^T staging)
        #   tag "ps_B" [128,512]  x4 = 4 banks  (QKV transients, Y' accum, proj)
        const_cm = tc.tile_pool(name="const", bufs=1)
        work_cm = tc.tile_pool(name="work", bufs=1)
        sb_cm = tc.tile_pool(name="sbw", bufs=2)
        ps_cm = tc.tile_pool(name="psum", bufs=1, space="PSUM")
        const = const_cm.__enter__()
        work = work_cm.__enter__()
        sbw = sb_cm.__enter__()
        psp = ps_cm.__enter__()

        def body(_i=None):
            # ---- constant + input loads ----
            wq_sb = [const.tile([P, 384], BF16, tag=f"wq{k}", name=f"wq{k}") for k in range(CKC)]
            wk_sb = [const.tile([P, 384], BF16, tag=f"wk{k}", name=f"wk{k}") for k in range(CKC)]
            wv_sb = [const.tile([P, 390], BF16, tag=f"wv{k}", name=f"wv{k}") for k in range(CKC)]
            w2_sb = [const.tile([P, C], BF16, tag=f"w2{p}", name=f"w2{p}") for p in range(3)]
            bq_sb = const.tile([P, 3], F32, tag="bq")
            bk_sb = const.tile([P, 3], F32, tag="bk")
            bvb_sb = const.tile([P, 390], BF16, tag="bvb")
            mask_sb = const.tile([P, P], BF16, tag="mask")
            xt_sb = [work.tile([P, T], BF16, tag=f"xt{k}", name=f"xt{k}") for k in range(CKC)]
            # Emission order = scheduling priority.  Interleave xt/wk/wq
            # chunk-wise so the first QKV accumulation chain can start after
            # the first couple of DMAs instead of after the whole input set.
            for k in range(CKC):
                nc.sync.dma_start(xt_sb[k][:, 0:512], d_xt[k * P:(k + 1) * P, 0:512])
                nc.sync.dma_start(wk_sb[k][:], d_wk[k * P:(k + 1) * P, :])
                nc.sync.dma_start(wq_sb[k][:], d_wq[k * P:(k + 1) * P, :])
            nc.sync.dma_start(bq_sb[:], d_bq[:])
            nc.sync.dma_start(bk_sb[:], d_bk[:])
            for k in range(CKC):
                nc.sync.dma_start(wv_sb[k][:], d_wv[k * P:(k + 1) * P, :])
            nc.sync.dma_start(bvb_sb[:], d_bvb[:])
            nc.sync.dma_start(mask_sb[:], d_mask[:])
            for k in range(CKC):
                nc.sync.dma_start(
                    xt_sb[k][:, 512:T], d_xt[k * P:(k + 1) * P, 512:T])
            for p in range(3):
                nc.sync.dma_start(w2_sb[p][:], d_w2[p * P:(p + 1) * P, :])

            qt_sb = [work.tile([P, T], BF16, tag=f"qt{p}", name=f"qtp{p}") for p in range(3)]
            kt_sb = [work.tile([P, T], BF16, tag=f"kt{p}", name=f"ktp{p}") for p in range(3)]
            v_sb = [work.tile([P, 390], BF16, tag=f"v{t}", name=f"v{t}") for t in range(KC_N)]
            yn_sb = [work.tile([P, T], BF16, tag=f"yn{p}", name=f"yn{p}") for p in range(3)]

            # ---- stage A(j): kt/qt column j, V chunks 4j..4j+3 ----
            # Returns emission closures, one per PSUM group.
            def a_qk(j, p):
                ops = []
                for (w_sb, b_sb, o_sb) in ((wk_sb, bk_sb, kt_sb), (wq_sb, bq_sb, qt_sb)):
                    def qk_group(p=p, w_sb=w_sb, b_sb=b_sb, o_sb=o_sb):
                        ps = psp.tile([P, 512], F32, tag="ps_B", bufs=2)
                        for k in range(CKC):
                            nc.tensor.matmul(
                                ps[:],
                                lhsT=w_sb[k][:, p * P:(p + 1) * P],
                                rhs=xt_sb[k][:, j * 512:(j + 1) * 512],
                                start=(k == 0),
                                stop=(k == CKC - 1),
                            )
                        nc.vector.tensor_scalar(
                            o_sb[p][:, j * 512:(j + 1) * 512],
                            ps[:],
                            b_sb[:, p:p + 1],
                            None,
                            mybir.AluOpType.add,
                        )
                    ops.append(qk_group)
                return ops

            def a_v(j):
                ops = []
                for t in range(4 * j, 4 * j + 4):
                    def v_group(t=t):
                        ps = psp.tile([P, 512], F32, tag="ps_B", bufs=2)
                        for k in range(CKC):
                            nc.tensor.matmul(
                                ps[:, :390],
                                lhsT=xt_sb[k][:, t * P:(t + 1) * P],
                                rhs=wv_sb[k][:],
                                start=(k == 0),
                                stop=(k == CKC - 1),
                            )
                        nc.vector.tensor_tensor(
                            v_sb[t][:], ps[:, :390], bvb_sb[:],
                            mybir.AluOpType.add,
                        )
                    ops.append(v_group)
                return ops

            def stage_a(j):
                return (a_qk(j, 0) + a_qk(j, 1) + a_qk(j, 2) + a_v(j))

            # ---- stage C(j): output projection for q-chunk j ----
            def stage_c(j):
                ops = []
                for qb in range(4 * j, 4 * j + 4):
                    def proj_group(qb=qb):
                        po_hi = psp.tile([P, 512], F32, tag="ps_B", bufs=2,
                                         name=f"poh{qb}")
                        po_lo = psp.tile([P, 512], F32, tag="ps_B", bufs=2,
                                         name=f"pol{qb}")
                        for (tile_, n0, nw) in ((po_hi, 0, 512), (po_lo, 512, 256)):
                            for pp in range(3):
                                nc.tensor.matmul(
                                    tile_[:, :nw],
                                    lhsT=yn_sb[pp][:, qb * P:(qb + 1) * P],
                                    rhs=w2_sb[pp][:, n0:n0 + nw],
                                    start=(pp == 0),
                                    stop=(pp == 2),
                                )
                        # SBUF staging on the lightly-loaded GpSimd engine
                        ob = sbw.tile([P, C], F32R, tag="ob", bufs=3)
                        nc.vector.tensor_copy(ob[:, 0:512], po_hi[:, :512])
                        nc.vector.tensor_copy(ob[:, 512:768], po_lo[:, :256])
                        nc.sync.dma_start(d_out[qb * P:(qb + 1) * P, :], ob[:])
                    ops.append(proj_group)
                return ops

            # ---- stage B(j): causal attention for q-chunk j ----
            # filler: list of closures drained one per kc iteration.
            def stage_b(j, filler):
                n_kc = 4 * j + 4
                fi = 0
                for p in range(3):
                    yps = [psp.tile([P, 512], F32, tag="ps_Y", bufs=2,
                                    name=f"yp{j}{p}{h2}") for h2 in range(2)]
                    # one-iteration lookahead: S(kc) is emitted before
                    # PV(kc-1) so the PE queue never head-blocks on exp(kc-1)
                    pend_pv = None
                    for kc in range(n_kc):
                        m = kc - 4 * j
                        # live q-cols of this 512-chunk start at 128*m
                        s0 = 128 * m if m > 0 else 0
                        ss = psp.tile([P, 1024], F32, tag="ps_A", bufs=2)
                        for h2 in range(2):
                            pb = 64 * h2
                            nc.tensor.matmul(
                                ss[:, h2 * 512 + s0:(h2 + 1) * 512],
                                lhsT=kt_sb[p][pb:pb + 64, kc * P:(kc + 1) * P],
                                rhs=qt_sb[p][pb:pb + 64,
                                             j * 512 + s0:(j + 1) * 512],
                                start=True,
                                stop=True,
                            )
                        pt = sbw.tile([P, 1024], BF16, tag="pt", bufs=4)
                        if s0:
                            ss_r = ss.rearrange("p (h c) -> p h c", h=2)
                            pt_r = pt.rearrange("p (h c) -> p h c", h=2)
                            nc.scalar.activation(
                                pt_r[:, :, s0:], ss_r[:, :, s0:],
                                mybir.ActivationFunctionType.Exp,
                            )
                        else:
                            nc.scalar.activation(
                                pt[:], ss[:], mybir.ActivationFunctionType.Exp
                            )
                        if m >= 0:
                            # mask the 128-wide diagonal triangle only
                            for h2 in range(2):
                                c0 = h2 * 512 + 128 * m
                                nc.vector.tensor_tensor(
                                    pt[:, c0:c0 + P],
                                    pt[:, c0:c0 + P],
                                    mask_sb[:],
                                    mybir.AluOpType.mult,
                                )
                        if pend_pv is not None:
                            pend_pv()
                        if fi < len(filler):
                            filler[fi]()
                            fi += 1

                        def pv(kc=kc, s0=s0, pt=pt):
                            for h2 in range(2):
                                ch = p * 2 + h2
                                nc.tensor.matmul(
                                    yps[h2][:65, s0:],
                                    lhsT=v_sb[kc][:, ch * 65:(ch + 1) * 65],
                                    rhs=pt[:, h2 * 512 + s0:(h2 + 1) * 512],
                                    start=(kc == 0),
                                    stop=(kc == n_kc - 1),
                                )
                        pend_pv = pv
                    pend_pv()
                    # normalize: divide rows 0..63 by denominator row 64.
                    # Approx DVE reciprocal (custom op must read SBUF, not
                    # PSUM on HW); the PSUM->SBUF denominator copy rides the
                    # otherwise-idle GpSimd engine.
                    for h2 in range(2):
                        pb = 64 * h2
                        den = sbw.tile([1, 512], F32, tag="den", bufs=2)
                        nc.scalar.copy(den[:], yps[h2][64:65, :])
                        recip = sbw.tile([1, 512], F32, tag="recip", bufs=2)
                        nc.vector.reciprocal_approx_fast(recip[:], den[:])
                        bcs = sbw.tile([64, 512], F32, tag="bcs", bufs=2)
                        nc.gpsimd.partition_broadcast(bcs[:], recip[:])
                        nc.vector.tensor_tensor(
                            yn_sb[p][pb:pb + 64, j * 512:(j + 1) * 512],
                            yps[h2][:64, :],
                            bcs[:],
                            mybir.AluOpType.mult,
                        )
                # drain any remaining filler
                while fi < len(filler):
                    filler[fi]()
                    fi += 1

            # ---- pipeline ----
            # Pre-roll: only what B(0) p=0 kc=0 needs; the rest of A(0)
            # leads B(0)'s filler queue.  A(3)'s V groups are only consumed
            # late in B(3), so they fill B(3)'s scalar-heavy tail along with
            # the projections of earlier q-chunks.
            av0 = a_v(0)
            for op in a_qk(0, 0) + av0[:2]:
                op()
            stage_b(0, av0[2:] + a_qk(0, 1) + a_qk(0, 2) + stage_a(1))
            stage_b(1, stage_c(0) + stage_a(2))
            stage_b(2, stage_c(1) + a_qk(3, 0) + a_qk(3, 1) + a_qk(3, 2))
            stage_b(3, a_v(3) + stage_c(2))
            for op in stage_c(3):
                op()

        if n_iters == 1:
            body()
        else:
            with tc.For_i(0, n_iters, 1) as _i:
                body(_i)

        for cm in (ps_cm, sb_cm, work_cm, const_cm):
            cm.__exit__(None, None, None)

    nc.compile()
    return nc


def shard_inputs(x, W_attn, b_attn, W_proj, b_proj, **_compat):
    """Builds the 8 per-core input maps (all host-side numpy prep)."""
    import ml_dtypes

    x = np.asarray(x, dtype=np.float32)
    W_attn = np.asarray(W_attn, dtype=np.float32)
    b_attn = np.asarray(b_attn, dtype=np.float32)
    W_proj = np.asarray(W_proj, dtype=np.float32)
    scale = float(HD) ** -0.5
    bf16 = ml_dtypes.bfloat16

    kl = np.arange(P)[:, None]
    ql = np.arange(P)[None, :]
    mask = (kl <= ql).astype(np.float32).astype(bf16)  # [128,128] triangle

    in_maps = []
    for core in range(N_CORES):
        b = core // 2
        s = core % 2
        heads = [s * HPC + j for j in range(HPC)]
        xt = np.ascontiguousarray(x[b].T).astype(bf16)  # [C, T]

        wq = np.empty((C, 384), np.float32)
        wk = np.empty((C, 384), np.float32)
        bq = np.empty((P, 3), np.float32)
        bk = np.empty((P, 3), np.float32)
        for p in range(3):
            for h2 in range(2):
                hh = heads[p * 2 + h2]
                cols = slice(hh * HD, (hh + 1) * HD)
                dst = slice(h2 * HD, (h2 + 1) * HD)
                wq[:, p * P + h2 * HD:p * P + (h2 + 1) * HD] = (
                    W_attn[:, cols] * scale
                )
                wk[:, p * P + h2 * HD:p * P + (h2 + 1) * HD] = (
                    W_attn[:, C + hh * HD:C + (hh + 1) * HD]
                )
                bq[dst, p] = b_attn[hh * HD:(hh + 1) * HD] * scale
                bk[dst, p] = b_attn[C + hh * HD:C + (hh + 1) * HD]

        wv = np.zeros((C, 390), np.float32)
        bv = np.zeros((1, 390), np.float32)
        for ch in range(HPC):
            hh = heads[ch]
            wv[:, ch * 65:ch * 65 + HD] = W_attn[:, 2 * C + hh * HD:2 * C + (hh + 1) * HD]
            bv[0, ch * 65:ch * 65 + HD] = b_attn[2 * C + hh * HD:2 * C + (hh + 1) * HD]
            bv[0, ch * 65 + HD] = 1.0
        bvb = np.broadcast_to(bv, (P, 390)).copy()

        w2 = np.empty((384, C), np.float32)
        for p in range(3):
            for h2 in range(2):
                hh = heads[p * 2 + h2]
                w2[p * P + h2 * HD:p * P + (h2 + 1) * HD, :] = (
                    W_proj[hh * HD:(hh + 1) * HD, :]
                )

        in_maps.append({
            "xt": xt,
            "wq": wq.astype(bf16), "wk": wk.astype(bf16),
            "wv": wv.astype(bf16), "w2": w2.astype(bf16),
            "bq": bq, "bk": bk, "bvb": bvb.astype(bf16),
            "masks": mask,
        })
    return in_maps


def unshard_outputs(results, b_proj):
    b_proj = np.asarray(b_proj, dtype=np.float32)
    out = np.empty((B, T, C), np.float32)
    for b in range(B):
        out[b] = results[2 * b]["out"] + results[2 * b + 1]["out"] + b_proj
    return out


_CACHED_NC = None


def kernel(x, W_attn, b_attn, W_proj, b_proj):
    global _CACHED_NC
    from concourse import bass_utils

    if _CACHED_NC is None:
        _CACHED_NC = build_program(1)
    in_maps = shard_inputs(x, W_attn, b_attn, W_proj, b_proj)
    res = bass_utils.run_bass_kernel_spmd(
        _CACHED_NC, in_maps, core_ids=list(range(N_CORES))
    )
    return unshard_outputs(res.results, b_proj)


# revision 18
# speedup vs baseline: 1.0271x; 1.0271x over previous
"""Causal self-attention (B=4, T=2048, C=768, 12 heads) on 8 Trainium2 cores.

Sharding: core i handles batch b = i//2 and head-set s = i%2 (6 of 12 heads).
Each core computes x[b] @ W_attn slice -> 6 heads of causal attention -> a
partial projection (row-sharded W_proj).  The host sums the two partials per
batch and adds b_proj.

v2: software-pipelined emission.  The kernel is organized as 4 k-column
groups j=0..3 (512 T-positions each).  Stage A(j) computes kt/qt column j
and V chunks 4j..4j+3; stage B(j) runs causal attention for q-chunk j over
k-chunks 0..4j+3; stage C(j) projects q-chunk j.  A(j+1) and C(j-1) matmul
groups are interleaved into B(j)'s kc loop as PE filler so the ScalarE exp
stream (the bottleneck engine, ~1.1us per kc tile) is never the only
runnable work: PE executes its queue in order, so emission order is the
schedule.

All matmul operands are bf16 (FWL-enabled weight loads, half SBUF/DMA
traffic); PSUM accumulation stays fp32.  Per-head layout as v1: Q^T/K^T in
[128, T] "pair" tiles (head a on partitions 0-63, head b on 64-127,
1/sqrt(64) folded into W_q/b_q), V' [T, 6*65] with a ones-column per head so
the PV matmul emits the softmax denominator row, S^T per (pair, q-chunk 512,
k-chunk 128) via row-tiled concurrent 64-contraction matmuls, exp on
ScalarE straight out of PSUM, diagonal tiles shrunk to live columns
(s0 = 128*m) with a single [128,128] triangle mask multiply on VectorE.
Normalization uses the fast approximate DVE reciprocal (~5x the ~3.3us
exact op).
"""

import numpy as np

import concourse.bass as bass
import concourse.mybir as mybir
import concourse.tile as tile
from concourse import bacc

B, T, C = 4, 2048, 768
NH, HD = 12, 64
N_CORES = 8
HPC = 6  # heads per core
P = 128
F32 = mybir.dt.float32
F32R = mybir.dt.float32r
BF16 = mybir.dt.bfloat16
QC_N = T // 512  # 4 q-chunks of 512
KC_N = T // P    # 16 k-chunks of 128
CKC = C // P     # 6 contraction chunks for the QKV projection


def build_program(n_iters: int = 1, **_compat):
    """Builds the SPMD program (identical on all cores; data differs)."""
    nc = bacc.Bacc(
        "TRN2",
        target_bir_lowering=False,
        debug=False,
        enable_asserts=False,
        num_devices=N_CORES,
    )
    d_xt = nc.dram_tensor("xt", [C, T], BF16, kind="ExternalInput").ap()
    d_wq = nc.dram_tensor("wq", [C, 384], BF16, kind="ExternalInput").ap()
    d_wk = nc.dram_tensor("wk", [C, 384], BF16, kind="ExternalInput").ap()
    d_wv = nc.dram_tensor("wv", [C, 390], BF16, kind="ExternalInput").ap()
    d_w2 = nc.dram_tensor("w2", [384, C], BF16, kind="ExternalInput").ap()
    d_bq = nc.dram_tensor("bq", [P, 3], F32, kind="ExternalInput").ap()
    d_bk = nc.dram_tensor("bk", [P, 3], F32, kind="ExternalInput").ap()
    d_bvb = nc.dram_tensor("bvb", [P, 390], BF16, kind="ExternalInput").ap()
    d_mask = nc.dram_tensor("masks", [P, P], BF16, kind="ExternalInput").ap()
    d_out = nc.dram_tensor("out", [T, C], F32R, kind="ExternalOutput").ap()

    with tile.TileContext(nc) as tc:
        # PSUM budget (8 banks):
        #   tag "ps_A" [128,1024] x2 = 4 banks  (S^T staging)
        #   tag "ps_B" [128,512]  x4 = 4 banks  (QKV transients, Y' accum, proj)
        const_cm = tc.tile_pool(name="const", bufs=1)
        work_cm = tc.tile_pool(name="work", bufs=1)
        sb_cm = tc.tile_pool(name="sbw", bufs=2)
        ps_cm = tc.tile_pool(name="psum", bufs=1, space="PSUM")
        const = const_cm.__enter__()
        work = work_cm.__enter__()
        sbw = sb_cm.__enter__()
        psp = ps_cm.__enter__()

        def body(_i=None):
            # ---- constant + input loads ----
            wq_sb = [const.tile([P, 384], BF16, tag=f"wq{k}", name=f"wq{k}") for k in range(CKC)]
            wk_sb = [const.tile([P, 384], BF16, tag=f"wk{k}", name=f"wk{k}") for k in range(CKC)]
            wv_sb = [const.tile([P, 390], BF16, tag=f"wv{k}", name=f"wv{k}") for k in range(CKC)]
            w2_sb = [const.tile([P, C], BF16, tag=f"w2{p}", name=f"w2{p}") for p in range(3)]
            bq_sb = const.tile([P, 3], F32, tag="bq")
            bk_sb = const.tile([P, 3], F32, tag="bk")
            bvb_sb = const.tile([P, 390], BF16, tag="bvb")
            mask_sb = const.tile([P, P], BF16, tag="mask")
            xt_sb = [work.tile([P, T], BF16, tag=f"xt{k}", name=f"xt{k}") for k in range(CKC)]
            # Emission order = scheduling priority.  Interleave xt/wk/wq
            # chunk-wise so the first QKV accumulation chain can start after
            # the first couple of DMAs instead of after the whole input set.
            for k in range(CKC):
                nc.sync.dma_start(xt_sb[k][:, 0:512], d_xt[k * P:(k + 1) * P, 0:512])
                nc.sync.dma_start(wk_sb[k][:], d_wk[k * P:(k + 1) * P, :])
                nc.sync.dma_start(wq_sb[k][:], d_wq[k * P:(k + 1) * P, :])
            nc.sync.dma_start(bq_sb[:], d_bq[:])
            nc.sync.dma_start(bk_sb[:], d_bk[:])
            for k in range(CKC):
                nc.sync.dma_start(wv_sb[k][:], d_wv[k * P:(k + 1) * P, :])
            nc.sync.dma_start(bvb_sb[:], d_bvb[:])
            nc.sync.dma_start(mask_sb[:], d_mask[:])
            for k in range(CKC):
                nc.sync.dma_start(
                    xt_sb[k][:, 512:T], d_xt[k * P:(k + 1) * P, 512:T])
            for p in range(3):
                nc.sync.dma_start(w2_sb[p][:], d_w2[p * P:(p + 1) * P, :])

            qt_sb = [work.tile([P, T], BF16, tag=f"qt{p}", name=f"qtp{p}") for p in range(3)]
            kt_sb = [work.tile([P, T], BF16, tag=f"kt{p}", name=f"ktp{p}") for p in range(3)]
            v_sb = [work.tile([P, 390], BF16, tag=f"v{t}", name=f"v{t}") for t in range(KC_N)]
            yn_sb = [work.tile([P, T], BF16, tag=f"yn{p}", name=f"yn{p}") for p in range(3)]

            # ---- stage A(j): kt/qt column j, V chunks 4j..4j+3 ----
            # Returns emission closures, one per PSUM group.
            def a_qk(j, p):
                ops = []
                for (w_sb, b_sb, o_sb) in ((wk_sb, bk_sb, kt_sb), (wq_sb, bq_sb, qt_sb)):
                    def qk_group(p=p, w_sb=w_sb, b_sb=b_sb, o_sb=o_sb):
                        ps = psp.tile([P, 512], F32, tag="ps_B", bufs=2)
                        for k in range(CKC):
                            nc.tensor.matmul(
                                ps[:],
                                lhsT=w_sb[k][:, p * P:(p + 1) * P],
                                rhs=xt_sb[k][:, j * 512:(j + 1) * 512],
                                start=(k == 0),
                                stop=(k == CKC - 1),
                            )
                        nc.vector.tensor_scalar(
                            o_sb[p][:, j * 512:(j + 1) * 512],
                            ps[:],
                            b_sb[:, p:p + 1],
                            None,
                            mybir.AluOpType.add,
                        )
                    ops.append(qk_group)
                return ops

            def a_v(j):
                ops = []
                for t in range(4 * j, 4 * j + 4):
                    def v_group(t=t):
                        ps = psp.tile([P, 512], F32, tag="ps_B", bufs=2)
                        for k in range(CKC):
                            nc.tensor.matmul(
                                ps[:, :390],
                                lhsT=xt_sb[k][:, t * P:(t + 1) * P],
                                rhs=wv_sb[k][:],
                                start=(k == 0),
                                stop=(k == CKC - 1),
                            )
                        nc.vector.tensor_tensor(
                            v_sb[t][:], ps[:, :390], bvb_sb[:],
                            mybir.AluOpType.add,
                        )
                    ops.append(v_group)
                return ops

            def stage_a(j):
                return (a_qk(j, 0) + a_qk(j, 1) + a_qk(j, 2) + a_v(j))

            # ---- stage C(j): output projection for q-chunk j ----
            def stage_c(j):
                ops = []
                for qb in range(4 * j, 4 * j + 4):
                    def proj_group(qb=qb):
                        po_hi = psp.tile([P, 512], F32, tag="ps_B", bufs=2,
                                         name=f"poh{qb}")
                        po_lo = psp.tile([P, 512], F32, tag="ps_B", bufs=2,
                                         name=f"pol{qb}")
                        for (tile_, n0, nw) in ((po_hi, 0, 512), (po_lo, 512, 256)):
                            for pp in range(3):
                                nc.tensor.matmul(
                                    tile_[:, :nw],
                                    lhsT=yn_sb[pp][:, qb * P:(qb + 1) * P],
                                    rhs=w2_sb[pp][:, n0:n0 + nw],
                                    start=(pp == 0),
                                    stop=(pp == 2),
                                )
                        # SBUF staging on the lightly-loaded GpSimd engine
                        ob = sbw.tile([P, C], F32R, tag="ob", bufs=4)
                        nc.vector.tensor_copy(ob[:, 0:512], po_hi[:, :512])
                        nc.vector.tensor_copy(ob[:, 512:768], po_lo[:, :256])
                        nc.sync.dma_start(d_out[qb * P:(qb + 1) * P, :], ob[:])
                    ops.append(proj_group)
                return ops

            # ---- stage B(j): causal attention for q-chunk j ----
            # filler: list of closures drained one per kc iteration.
            def stage_b(j, filler, pairs=(0, 1, 2)):
                n_kc = 4 * j + 4
                fi = 0
                for p in pairs:
                    yps = [psp.tile([P, 512], F32, tag="ps_Y", bufs=2,
                                    name=f"yp{j}{p}{h2}") for h2 in range(2)]
                    # one-iteration lookahead: S(kc) is emitted before
                    # PV(kc-1) so the PE queue never head-blocks on exp(kc-1)
                    pend_pv = None
                    for kc in range(n_kc):
                        m = kc - 4 * j
                        # live q-cols of this 512-chunk start at 128*m
                        s0 = 128 * m if m > 0 else 0
                        ss = psp.tile([P, 1024], F32, tag="ps_A", bufs=2)
                        for h2 in range(2):
                            pb = 64 * h2
                            nc.tensor.matmul(
                                ss[:, h2 * 512 + s0:(h2 + 1) * 512],
                                lhsT=kt_sb[p][pb:pb + 64, kc * P:(kc + 1) * P],
                                rhs=qt_sb[p][pb:pb + 64,
                                             j * 512 + s0:(j + 1) * 512],
                                start=True,
                                stop=True,
                            )
                        pt = sbw.tile([P, 1024], BF16, tag="pt", bufs=6)
                        if s0:
                            ss_r = ss.rearrange("p (h c) -> p h c", h=2)
                            pt_r = pt.rearrange("p (h c) -> p h c", h=2)
                            nc.scalar.activation(
                                pt_r[:, :, s0:], ss_r[:, :, s0:],
                                mybir.ActivationFunctionType.Exp,
                            )
                        else:
                            nc.scalar.activation(
                                pt[:], ss[:], mybir.ActivationFunctionType.Exp
                            )
                        if m >= 0:
                            # mask the 128-wide diagonal triangle only
                            for h2 in range(2):
                                c0 = h2 * 512 + 128 * m
                                nc.vector.tensor_tensor(
                                    pt[:, c0:c0 + P],
                                    pt[:, c0:c0 + P],
                                    mask_sb[:],
                                    mybir.AluOpType.mult,
                                )
                        if pend_pv is not None:
                            pend_pv()
                        if fi < len(filler):
                            filler[fi]()
                            fi += 1

                        def pv(kc=kc, s0=s0, pt=pt):
                            for h2 in range(2):
                                ch = p * 2 + h2
                                nc.tensor.matmul(
                                    yps[h2][:65, s0:],
                                    lhsT=v_sb[kc][:, ch * 65:(ch + 1) * 65],
                                    rhs=pt[:, h2 * 512 + s0:(h2 + 1) * 512],
                                    start=(kc == 0),
                                    stop=(kc == n_kc - 1),
                                )
                        pend_pv = pv
                    pend_pv()
                    # normalize: divide rows 0..63 by denominator row 64.
                    # Approx DVE reciprocal (custom op must read SBUF, not
                    # PSUM on HW); the PSUM->SBUF denominator copy rides the
                    # otherwise-idle GpSimd engine.
                    for h2 in range(2):
                        pb = 64 * h2
                        den = sbw.tile([1, 512], F32, tag="den", bufs=4)
                        nc.scalar.copy(den[:], yps[h2][64:65, :])
                        recip = sbw.tile([1, 512], F32, tag="recip", bufs=4)
                        nc.vector.reciprocal_approx_fast(recip[:], den[:])
                        bcs = sbw.tile([64, 512], F32, tag="bcs", bufs=4)
                        nc.gpsimd.partition_broadcast(bcs[:], recip[:])
                        nc.vector.tensor_tensor(
                            yn_sb[p][pb:pb + 64, j * 512:(j + 1) * 512],
                            yps[h2][:64, :],
                            bcs[:],
                            mybir.AluOpType.mult,
                        )
                # drain any remaining filler
                while fi < len(filler):
                    filler[fi]()
                    fi += 1

            # ---- pipeline ----
            # Pre-roll: only what B(0) p=0 kc=0 needs; the rest of A(0)
            # leads B(0)'s filler queue.  A(3)'s V groups are only consumed
            # late in B(3), so they fill B(3)'s scalar-heavy tail along with
            # the projections of earlier q-chunks.
            av0 = a_v(0)
            for op in a_qk(0, 0) + av0[:2]:
                op()
            stage_b(0, av0[2:] + a_qk(0, 1) + a_qk(0, 2) + stage_a(1))
            stage_b(1, stage_c(0) + stage_a(2))
            # B(3) is exp-heavy (16 k-chunks/pair) with little PE work left
            # to fill; pull pair 0 of it forward so the scalar-bound tail
            # only spans two pairs.
            c1 = stage_c(1)
            c2 = stage_c(2)
            stage_b(2, a_qk(3, 0) + a_v(3) + a_qk(3, 1) + a_qk(3, 2) + c1)
            stage_b(3, c2[:2], pairs=(0,))
            stage_b(3, c2[2:], pairs=(1, 2))
            for op in stage_c(3):
                op()

        if n_iters == 1:
            body()
        else:
            with tc.For_i(0, n_iters, 1) as _i:
                body(_i)

        for cm in (ps_cm, sb_cm, work_cm, const_cm):
            cm.__exit__(None, None, None)

    nc.compile()
    return nc


def shard_inputs(x, W_attn, b_attn, W_proj, b_proj, **_compat):
    """Builds the 8 per-core input maps (all host-side numpy prep)."""
    import ml_dtypes

    x = np.asarray(x, dtype=np.float32)
    W_attn = np.asarray(W_attn, dtype=np.float32)
    b_attn = np.asarray(b_attn, dtype=np.float32)
    W_proj = np.asarray(W_proj, dtype=np.float32)
    scale = float(HD) ** -0.5
    bf16 = ml_dtypes.bfloat16

    kl = np.arange(P)[:, None]
    ql = np.arange(P)[None, :]
    mask = (kl <= ql).astype(np.float32).astype(bf16)  # [128,128] triangle

    in_maps = []
    for core in range(N_CORES):
        b = core // 2
        s = core % 2
        heads = [s * HPC + j for j in range(HPC)]
        xt = np.ascontiguousarray(x[b].T).astype(bf16)  # [C, T]

        wq = np.empty((C, 384), np.float32)
        wk = np.empty((C, 384), np.float32)
        bq = np.empty((P, 3), np.float32)
        bk = np.empty((P, 3), np.float32)
        for p in range(3):
            for h2 in range(2):
                hh = heads[p * 2 + h2]
                cols = slice(hh * HD, (hh + 1) * HD)
                dst = slice(h2 * HD, (h2 + 1) * HD)
                wq[:, p * P + h2 * HD:p * P + (h2 + 1) * HD] = (
                    W_attn[:, cols] * scale
                )
                wk[:, p * P + h2 * HD:p * P + (h2 + 1) * HD] = (
                    W_attn[:, C + hh * HD:C + (hh + 1) * HD]
                )
                bq[dst, p] = b_attn[hh * HD:(hh + 1) * HD] * scale
                bk[dst, p] = b_attn[C + hh * HD:C + (hh + 1) * HD]

        wv = np.zeros((C, 390), np.float32)
        bv = np.zeros((1, 390), np.float32)
        for ch in range(HPC):
            hh = heads[ch]
            wv[:, ch * 65:ch * 65 + HD] = W_attn[:, 2 * C + hh * HD:2 * C + (hh + 1) * HD]
            bv[0, ch * 65:ch * 65 + HD] = b_attn[2 * C + hh * HD:2 * C + (hh + 1) * HD]
            bv[0, ch * 65 + HD] = 1.0
        bvb = np.broadcast_to(bv, (P, 390)).copy()

        w2 = np.empty((384, C), np.float32)
        for p in range(3):
            for h2 in range(2):
                hh = heads[p * 2 + h2]
                w2[p * P + h2 * HD:p * P + (h2 + 1) * HD, :] = (
                    W_proj[hh * HD:(hh + 1) * HD, :]
                )

        in_maps.append({
            "xt": xt,
            "wq": wq.astype(bf16), "wk": wk.astype(bf16),
            "wv": wv.astype(bf16), "w2": w2.astype(bf16),
            "bq": bq, "bk": bk, "bvb": bvb.astype(bf16),
            "masks": mask,
        })
    return in_maps


def unshard_outputs(results, b_proj):
    b_proj = np.asarray(b_proj, dtype=np.float32)
    out = np.empty((B, T, C), np.float32)
    for b in range(B):
        out[b] = results[2 * b]["out"] + results[2 * b + 1]["out"] + b_proj
    return out


_CACHED_NC = None


def kernel(x, W_attn, b_attn, W_proj, b_proj):
    global _CACHED_NC
    from concourse import bass_utils

    if _CACHED_NC is None:
        _CACHED_NC = build_program(1)
    in_maps = shard_inputs(x, W_attn, b_attn, W_proj, b_proj)
    res = bass_utils.run_bass_kernel_spmd(
        _CACHED_NC, in_maps, core_ids=list(range(N_CORES))
    )
    return unshard_outputs(res.results, b_proj)


# revision 20
# speedup vs baseline: 1.0475x; 1.0198x over previous
"""Causal self-attention (B=4, T=2048, C=768, 12 heads) on 8 Trainium2 cores.

Sharding: core i handles batch b = i//2 and head-set s = i%2 (6 of 12 heads).
Each core computes x[b] @ W_attn slice -> 6 heads of causal attention -> a
partial projection (row-sharded W_proj).  The host sums the two partials per
batch and adds b_proj.

v2: software-pipelined emission.  The kernel is organized as 4 k-column
groups j=0..3 (512 T-positions each).  Stage A(j) computes kt/qt column j
and V chunks 4j..4j+3; stage B(j) runs causal attention for q-chunk j over
k-chunks 0..4j+3; stage C(j) projects q-chunk j.  A(j+1) and C(j-1) matmul
groups are interleaved into B(j)'s kc loop as PE filler so the ScalarE exp
stream (the bottleneck engine, ~1.1us per kc tile) is never the only
runnable work: PE executes its queue in order, so emission order is the
schedule.

All matmul operands are bf16 (FWL-enabled weight loads, half SBUF/DMA
traffic); PSUM accumulation stays fp32.  Per-head layout as v1: Q^T/K^T in
[128, T] "pair" tiles (head a on partitions 0-63, head b on 64-127,
1/sqrt(64) folded into W_q/b_q), V' [T, 6*65] with a ones-column per head so
the PV matmul emits the softmax denominator row, S^T per (pair, q-chunk 512,
k-chunk 128) via row-tiled concurrent 64-contraction matmuls, exp on
ScalarE straight out of PSUM, diagonal tiles shrunk to live columns
(s0 = 128*m) with a single [128,128] triangle mask multiply on VectorE.
Normalization uses the fast approximate DVE reciprocal (~5x the ~3.3us
exact op).
"""

import numpy as np

import concourse.bass as bass
import concourse.mybir as mybir
import concourse.tile as tile
from concourse import bacc

B, T, C = 4, 2048, 768
NH, HD = 12, 64
N_CORES = 8
HPC = 6  # heads per core
P = 128
F32 = mybir.dt.float32
F32R = mybir.dt.float32r
BF16 = mybir.dt.bfloat16
QC_N = T // 512  # 4 q-chunks of 512
KC_N = T // P    # 16 k-chunks of 128
CKC = C // P     # 6 contraction chunks for the QKV projection


def build_program(n_iters: int = 1, **_compat):
    """Builds the SPMD program (identical on all cores; data differs)."""
    nc = bacc.Bacc(
        "TRN2",
        target_bir_lowering=False,
        debug=False,
        enable_asserts=False,
        num_devices=N_CORES,
    )
    d_xt = nc.dram_tensor("xt", [C, T], BF16, kind="ExternalInput").ap()
    d_wq = nc.dram_tensor("wq", [C, 384], BF16, kind="ExternalInput").ap()
    d_wk = nc.dram_tensor("wk", [C, 384], BF16, kind="ExternalInput").ap()
    d_wv = nc.dram_tensor("wv", [C, 390], BF16, kind="ExternalInput").ap()
    d_w2 = nc.dram_tensor("w2", [384, C], BF16, kind="ExternalInput").ap()
    d_bq = nc.dram_tensor("bq", [P, 3], F32, kind="ExternalInput").ap()
    d_bk = nc.dram_tensor("bk", [P, 3], F32, kind="ExternalInput").ap()
    d_bvb = nc.dram_tensor("bvb", [P, 390], BF16, kind="ExternalInput").ap()
    d_mask = nc.dram_tensor("masks", [P, P], BF16, kind="ExternalInput").ap()
    d_out = nc.dram_tensor("out", [T, C], BF16, kind="ExternalOutput").ap()

    with tile.TileContext(nc) as tc:
        # PSUM budget (8 banks):
        #   tag "ps_A" [128,1024] x2 = 4 banks  (S^T staging)
        #   tag "ps_B" [128,512]  x4 = 4 banks  (QKV transients, Y' accum, proj)
        const_cm = tc.tile_pool(name="const", bufs=1)
        work_cm = tc.tile_pool(name="work", bufs=1)
        sb_cm = tc.tile_pool(name="sbw", bufs=2)
        ps_cm = tc.tile_pool(name="psum", bufs=1, space="PSUM")
        const = const_cm.__enter__()
        work = work_cm.__enter__()
        sbw = sb_cm.__enter__()
        psp = ps_cm.__enter__()

        def body(_i=None):
            # ---- constant + input loads ----
            wq_sb = [const.tile([P, 384], BF16, tag=f"wq{k}", name=f"wq{k}") for k in range(CKC)]
            wk_sb = [const.tile([P, 384], BF16, tag=f"wk{k}", name=f"wk{k}") for k in range(CKC)]
            wv_sb = [const.tile([P, 390], BF16, tag=f"wv{k}", name=f"wv{k}") for k in range(CKC)]
            w2_sb = [const.tile([P, C], BF16, tag=f"w2{p}", name=f"w2{p}") for p in range(3)]
            bq_sb = const.tile([P, 3], F32, tag="bq")
            bk_sb = const.tile([P, 3], F32, tag="bk")
            bvb_sb = const.tile([P, 390], BF16, tag="bvb")
            mask_sb = const.tile([P, P], BF16, tag="mask")
            xt_sb = [work.tile([P, T], BF16, tag=f"xt{k}", name=f"xt{k}") for k in range(CKC)]
            # Emission order = scheduling priority.  Interleave xt/wk/wq
            # chunk-wise so the first QKV accumulation chain can start after
            # the first couple of DMAs instead of after the whole input set.
            for k in range(CKC):
                nc.sync.dma_start(xt_sb[k][:, 0:512], d_xt[k * P:(k + 1) * P, 0:512])
                nc.sync.dma_start(wk_sb[k][:], d_wk[k * P:(k + 1) * P, :])
                nc.sync.dma_start(wq_sb[k][:], d_wq[k * P:(k + 1) * P, :])
            nc.sync.dma_start(bq_sb[:], d_bq[:])
            nc.sync.dma_start(bk_sb[:], d_bk[:])
            for k in range(CKC):
                nc.sync.dma_start(wv_sb[k][:], d_wv[k * P:(k + 1) * P, :])
            nc.sync.dma_start(bvb_sb[:], d_bvb[:])
            nc.sync.dma_start(mask_sb[:], d_mask[:])
            for k in range(CKC):
                nc.sync.dma_start(
                    xt_sb[k][:, 512:T], d_xt[k * P:(k + 1) * P, 512:T])
            for p in range(3):
                nc.sync.dma_start(w2_sb[p][:], d_w2[p * P:(p + 1) * P, :])

            qt_sb = [work.tile([P, T], BF16, tag=f"qt{p}", name=f"qtp{p}") for p in range(3)]
            kt_sb = [work.tile([P, T], BF16, tag=f"kt{p}", name=f"ktp{p}") for p in range(3)]
            v_sb = [work.tile([P, 390], BF16, tag=f"v{t}", name=f"v{t}") for t in range(KC_N)]
            yn_sb = [work.tile([P, T], BF16, tag=f"yn{p}", name=f"yn{p}") for p in range(3)]

            # ---- stage A(j): kt/qt column j, V chunks 4j..4j+3 ----
            # Returns emission closures, one per PSUM group.
            def a_qk(j, p):
                ops = []
                for (w_sb, b_sb, o_sb) in ((wk_sb, bk_sb, kt_sb), (wq_sb, bq_sb, qt_sb)):
                    def qk_group(p=p, w_sb=w_sb, b_sb=b_sb, o_sb=o_sb):
                        ps = psp.tile([P, 512], F32, tag="ps_B", bufs=2)
                        for k in range(CKC):
                            nc.tensor.matmul(
                                ps[:],
                                lhsT=w_sb[k][:, p * P:(p + 1) * P],
                                rhs=xt_sb[k][:, j * 512:(j + 1) * 512],
                                start=(k == 0),
                                stop=(k == CKC - 1),
                            )
                        nc.vector.tensor_scalar(
                            o_sb[p][:, j * 512:(j + 1) * 512],
                            ps[:],
                            b_sb[:, p:p + 1],
                            None,
                            mybir.AluOpType.add,
                        )
                    ops.append(qk_group)
                return ops

            def a_v(j):
                ops = []
                for t in range(4 * j, 4 * j + 4):
                    def v_group(t=t):
                        ps = psp.tile([P, 512], F32, tag="ps_B", bufs=2)
                        for k in range(CKC):
                            nc.tensor.matmul(
                                ps[:, :390],
                                lhsT=xt_sb[k][:, t * P:(t + 1) * P],
                                rhs=wv_sb[k][:],
                                start=(k == 0),
                                stop=(k == CKC - 1),
                            )
                        nc.vector.tensor_tensor(
                            v_sb[t][:], ps[:, :390], bvb_sb[:],
                            mybir.AluOpType.add,
                        )
                    ops.append(v_group)
                return ops

            def stage_a(j):
                return (a_qk(j, 0) + a_qk(j, 1) + a_qk(j, 2) + a_v(j))

            # ---- stage C(j): output projection for q-chunk j ----
            def stage_c(j):
                ops = []
                for qb in range(4 * j, 4 * j + 4):
                    def proj_group(qb=qb):
                        po_hi = psp.tile([P, 512], F32, tag="ps_B", bufs=2,
                                         name=f"poh{qb}")
                        po_lo = psp.tile([P, 512], F32, tag="ps_B", bufs=2,
                                         name=f"pol{qb}")
                        for (tile_, n0, nw) in ((po_hi, 0, 512), (po_lo, 512, 256)):
                            for pp in range(3):
                                nc.tensor.matmul(
                                    tile_[:, :nw],
                                    lhsT=yn_sb[pp][:, qb * P:(qb + 1) * P],
                                    rhs=w2_sb[pp][:, n0:n0 + nw],
                                    start=(pp == 0),
                                    stop=(pp == 2),
                                )
                        # SBUF staging on the lightly-loaded GpSimd engine
                        ob = sbw.tile([P, C], BF16, tag="ob", bufs=4)
                        nc.vector.tensor_copy(ob[:, 0:512], po_hi[:, :512])
                        nc.vector.tensor_copy(ob[:, 512:768], po_lo[:, :256])
                        nc.sync.dma_start(d_out[qb * P:(qb + 1) * P, :], ob[:])
                    ops.append(proj_group)
                return ops

            # ---- stage B(j): causal attention for q-chunk j ----
            # filler: list of closures drained one per kc iteration.
            def stage_b(j, filler, pairs=(0, 1, 2)):
                n_kc = 4 * j + 4
                fi = 0
                for p in pairs:
                    yps = [psp.tile([P, 512], F32, tag="ps_Y", bufs=2,
                                    name=f"yp{j}{p}{h2}") for h2 in range(2)]
                    # one-iteration lookahead: S(kc) is emitted before
                    # PV(kc-1) so the PE queue never head-blocks on exp(kc-1)
                    pend_pv = None
                    for kc in range(n_kc):
                        m = kc - 4 * j
                        # live q-cols of this 512-chunk start at 128*m
                        s0 = 128 * m if m > 0 else 0
                        ss = psp.tile([P, 1024], F32, tag="ps_A", bufs=2)
                        for h2 in range(2):
                            pb = 64 * h2
                            nc.tensor.matmul(
                                ss[:, h2 * 512 + s0:(h2 + 1) * 512],
                                lhsT=kt_sb[p][pb:pb + 64, kc * P:(kc + 1) * P],
                                rhs=qt_sb[p][pb:pb + 64,
                                             j * 512 + s0:(j + 1) * 512],
                                start=True,
                                stop=True,
                            )
                        pt = sbw.tile([P, 1024], BF16, tag="pt", bufs=6)
                        if s0:
                            ss_r = ss.rearrange("p (h c) -> p h c", h=2)
                            pt_r = pt.rearrange("p (h c) -> p h c", h=2)
                            nc.scalar.activation(
                                pt_r[:, :, s0:], ss_r[:, :, s0:],
                                mybir.ActivationFunctionType.Exp,
                            )
                        else:
                            nc.scalar.activation(
                                pt[:], ss[:], mybir.ActivationFunctionType.Exp
                            )
                        if m >= 0:
                            # mask the 128-wide diagonal triangle only
                            for h2 in range(2):
                                c0 = h2 * 512 + 128 * m
                                nc.vector.tensor_tensor(
                                    pt[:, c0:c0 + P],
                                    pt[:, c0:c0 + P],
                                    mask_sb[:],
                                    mybir.AluOpType.mult,
                                )
                        if pend_pv is not None:
                            pend_pv()
                        if fi < len(filler):
                            filler[fi]()
                            fi += 1

                        def pv(kc=kc, s0=s0, pt=pt):
                            for h2 in range(2):
                                ch = p * 2 + h2
                                nc.tensor.matmul(
                                    yps[h2][:65, s0:],
                                    lhsT=v_sb[kc][:, ch * 65:(ch + 1) * 65],
                                    rhs=pt[:, h2 * 512 + s0:(h2 + 1) * 512],
                                    start=(kc == 0),
                                    stop=(kc == n_kc - 1),
                                )
                        pend_pv = pv
                    pend_pv()
                    # normalize: divide rows 0..63 by denominator row 64.
                    # Approx DVE reciprocal (custom op must read SBUF, not
                    # PSUM on HW); the PSUM->SBUF denominator copy rides the
                    # otherwise-idle GpSimd engine.
                    for h2 in range(2):
                        pb = 64 * h2
                        den = sbw.tile([1, 512], F32, tag="den", bufs=4)
                        nc.vector.tensor_copy(den[:], yps[h2][64:65, :])
                        recip = sbw.tile([1, 512], F32, tag="recip", bufs=4)
                        nc.vector.reciprocal_approx_fast(recip[:], den[:])
                        bcs = sbw.tile([64, 512], F32, tag="bcs", bufs=4)
                        nc.gpsimd.partition_broadcast(bcs[:], recip[:])
                        nc.vector.tensor_tensor(
                            yn_sb[p][pb:pb + 64, j * 512:(j + 1) * 512],
                            yps[h2][:64, :],
                            bcs[:],
                            mybir.AluOpType.mult,
                        )
                # drain any remaining filler
                while fi < len(filler):
                    filler[fi]()
                    fi += 1

            # ---- pipeline ----
            # Pre-roll: only what B(0) p=0 kc=0 needs; the rest of A(0)
            # leads B(0)'s filler queue.  A(3)'s V groups are only consumed
            # late in B(3), so they fill B(3)'s scalar-heavy tail along with
            # the projections of earlier q-chunks.
            av0 = a_v(0)
            for op in a_qk(0, 0) + av0[:2]:
                op()
            stage_b(0, av0[2:] + a_qk(0, 1) + a_qk(0, 2) + stage_a(1))
            stage_b(1, stage_c(0) + stage_a(2))
            # B(3) is exp-heavy (16 k-chunks/pair) with little PE work left
            # to fill; pull pair 0 of it forward so the scalar-bound tail
            # only spans two pairs.
            c1 = stage_c(1)
            c2 = stage_c(2)
            stage_b(2, a_qk(3, 0) + a_v(3) + a_qk(3, 1) + a_qk(3, 2) + c1)
            stage_b(3, c2[:2], pairs=(0,))
            stage_b(3, c2[2:], pairs=(1, 2))
            for op in stage_c(3):
                op()

        if n_iters == 1:
            body()
        else:
            with tc.For_i(0, n_iters, 1) as _i:
                body(_i)

        for cm in (ps_cm, sb_cm, work_cm, const_cm):
            cm.__exit__(None, None, None)

    nc.compile()
    return nc


def shard_inputs(x, W_attn, b_attn, W_proj, b_proj, **_compat):
    """Builds the 8 per-core input maps (all host-side numpy prep)."""
    import ml_dtypes

    x = np.asarray(x, dtype=np.float32)
    W_attn = np.asarray(W_attn, dtype=np.float32)
    b_attn = np.asarray(b_attn, dtype=np.float32)
    W_proj = np.asarray(W_proj, dtype=np.float32)
    scale = float(HD) ** -0.5
    bf16 = ml_dtypes.bfloat16

    kl = np.arange(P)[:, None]
    ql = np.arange(P)[None, :]
    mask = (kl <= ql).astype(np.float32).astype(bf16)  # [128,128] triangle

    in_maps = []
    for core in range(N_CORES):
        b = core // 2
        s = core % 2
        heads = [s * HPC + j for j in range(HPC)]
        xt = np.ascontiguousarray(x[b].T).astype(bf16)  # [C, T]

        wq = np.empty((C, 384), np.float32)
        wk = np.empty((C, 384), np.float32)
        bq = np.empty((P, 3), np.float32)
        bk = np.empty((P, 3), np.float32)
        for p in range(3):
            for h2 in range(2):
                hh = heads[p * 2 + h2]
                cols = slice(hh * HD, (hh + 1) * HD)
                dst = slice(h2 * HD, (h2 + 1) * HD)
                wq[:, p * P + h2 * HD:p * P + (h2 + 1) * HD] = (
                    W_attn[:, cols] * scale
                )
                wk[:, p * P + h2 * HD:p * P + (h2 + 1) * HD] = (
                    W_attn[:, C + hh * HD:C + (hh + 1) * HD]
                )
                bq[dst, p] = b_attn[hh * HD:(hh + 1) * HD] * scale
                bk[dst, p] = b_attn[C + hh * HD:C + (hh + 1) * HD]

        wv = np.zeros((C, 390), np.float32)
        bv = np.zeros((1, 390), np.float32)
        for ch in range(HPC):
            hh = heads[ch]
            wv[:, ch * 65:ch * 65 + HD] = W_attn[:, 2 * C + hh * HD:2 * C + (hh + 1) * HD]
            bv[0, ch * 65:ch * 65 + HD] = b_attn[2 * C + hh * HD:2 * C + (hh + 1) * HD]
            bv[0, ch * 65 + HD] = 1.0
        bvb = np.broadcast_to(bv, (P, 390)).copy()

        w2 = np.empty((384, C), np.float32)
        for p in range(3):
            for h2 in range(2):
                hh = heads[p * 2 + h2]
                w2[p * P + h2 * HD:p * P + (h2 + 1) * HD, :] = (
                    W_proj[hh * HD:(hh + 1) * HD, :]
                )

        in_maps.append({
            "xt": xt,
            "wq": wq.astype(bf16), "wk": wk.astype(bf16),
            "wv": wv.astype(bf16), "w2": w2.astype(bf16),
            "bq": bq, "bk": bk, "bvb": bvb.astype(bf16),
            "masks": mask,
        })
    return in_maps


def unshard_outputs(results, b_proj):
    b_proj = np.asarray(b_proj, dtype=np.float32)
    out = np.empty((B, T, C), np.float32)
    for b in range(B):
        out[b] = (results[2 * b]["out"].astype(np.float32)
                  + results[2 * b + 1]["out"].astype(np.float32) + b_proj)
    return out


_CACHED_NC = None


def kernel(x, W_attn, b_attn, W_proj, b_proj):
    global _CACHED_NC
    from concourse import bass_utils

    if _CACHED_NC is None:
        _CACHED_NC = build_program(1)
    in_maps = shard_inputs(x, W_attn, b_attn, W_proj, b_proj)
    res = bass_utils.run_bass_kernel_spmd(
        _CACHED_NC, in_maps, core_ids=list(range(N_CORES))
    )
    return unshard_outputs(res.results, b_proj)
